# revision 1
# baseline (speedup 1.0000x reference)
"""Distributed masked multi-head self-attention for Trainium2 (8 NeuronCores).

Problem: x:[2,2048,1024], mask:[2,2048], Wq:[1024,1024], Wkv:[1024,2048],
Wo:[1024,1024]  ->  out:[2,2048,1024]  (fp32)

Strategy (single SPMD launch, one NEFF on 8 cores):
  Phase 1 (head parallel): core c owns heads {2c, 2c+1} (128 contiguous
    columns of q/k/v).  Each core reads full x (transposed on host to
    [b, dim, n]) and computes qT/kT = [128, n] (head-dim major) and
    v = [n, 128] for its heads, then masked softmax attention entirely
    in "scores transposed" [key, query] layout:
      simT = kT_h slice as lhsT, qT as rhs -> [128 keys, q] in PSUM
      P    = exp(scale*simT + mask_bias_per_key_partition)   (ScalarE)
      outT[64h:, q] += v_h^T @ P  (col-tiled pairs, heads concurrent)
      denom[q]     += ones^T @ P  (col-tiled M=1 pairs)
    normalized att^T is [128 head-cols, 4096 rows] per core.
  AllToAll: block j of core c's att^T (rows 512j..512j+512 of the global
    row index) is exchanged so core c ends with att^T[:, rows_c] for ALL
    1024 columns -> [1024, 512] (row parallel).
  Phase 2 (row parallel): out_rows = att_rows @ Wo with full Wo; core c
    writes rows [512c, 512c+512).  Host concatenates.

Precision tiers (measured max rel err ~4e-3 vs fp64, HW ~325 us/call):
  - x, Wq/Wk/Wv:       bf16   (halves the dominant x-broadcast DMA)
  - q/k, scores, proj: float32r (fp32 storage, full PE rate, TF32-ish)
  - P=exp(s), v, att, Wo: bf16 (halves the serial AllToAll tail)
  - PSUM accumulation, softmax denominators, output: fp32
ATT_MM_MODE=f32 switches the f32r tier to exact fp32 (4x slower PE).
"""

import os
import sys

import numpy as np

for _p in ("/opt/trn_rl_repo",):
    if _p not in sys.path and os.path.isdir(_p):
        sys.path.append(_p)

import concourse.bass as bass
from concourse import bacc
import concourse.mybir as mybir
import concourse.tile as tile
from concourse.masks import make_identity
from contextlib import ExitStack

# ----- problem constants (hardcoded; kernel.py must be self-contained) -----
B, N, DIM, H, DH = 2, 2048, 1024, 16, 64
DI = H * DH                       # 1024
NCORES = 8
HPC = H // NCORES                 # 2 heads per core
CW = HPC * DH                     # 128 att columns per core
RPC = B * N // NCORES             # 512 output rows per core
SCALE = DH ** -0.5
MASK_NEG = -30000.0               # exp(scale*s + MASK_NEG) == 0 in fp32

P = 128                           # partitions
KT = DIM // P                     # 8 contraction tiles for projections
NKT = N // P                      # 16 key tiles
QW = 1024                         # query block width (exp batch width)
NQH = N // QW                     # 2 query blocks per batch
FP32 = mybir.dt.float32
F32R = mybir.dt.float32r
BF16 = mybir.dt.bfloat16

MM_MODE = os.environ.get("ATT_MM_MODE", "f32r")  # "f32r" | "f32"
# float32r: fp32 storage, full PE rate for free dim >= 256.  The BIR
# verifier requires every producer of an f32r-matmul operand to emit
# f32r, so those tensors are declared f32r end-to-end.
MDT = F32R if MM_MODE == "f32r" else FP32
PDT = BF16   # post-softmax path: P, v, att, Wo


def _r(ap):
    return ap


def build_program(reps=1):
    nc = bacc.Bacc(None, target_bir_lowering=False, num_devices=NCORES)

    xt = nc.dram_tensor("xt", [B, DIM, N], BF16, kind="ExternalInput")
    wq = nc.dram_tensor("wq", [DIM, CW], BF16, kind="ExternalInput")
    wk = nc.dram_tensor("wk", [DIM, CW], BF16, kind="ExternalInput")
    wv = nc.dram_tensor("wv", [DIM, CW], BF16, kind="ExternalInput")
    wo = nc.dram_tensor("wo", [DI, DIM], PDT, kind="ExternalInput")
    mb = nc.dram_tensor("mb", [B, P, NKT], FP32, kind="ExternalInput")
    out = nc.dram_tensor("out", [RPC, DIM], FP32, kind="ExternalOutput")

    with tile.TileContext(nc) as tc, ExitStack() as ctx, \
            nc.allow_low_precision(reason="f32r matmul pipeline; psum accum stays fp32"):
        const = ctx.enter_context(tc.tile_pool(name="const", bufs=1))
        wts = ctx.enter_context(tc.tile_pool(name="wts", bufs=1))
        xtp = ctx.enter_context(tc.tile_pool(name="xtp", bufs=12))
        qkp = ctx.enter_context(tc.tile_pool(name="qkp", bufs=2))
        vtp = ctx.enter_context(tc.tile_pool(name="vtp", bufs=1))
        vsp = ctx.enter_context(tc.tile_pool(name="vsp", bufs=2))
        pp = ctx.enter_context(tc.tile_pool(name="pp", bufs=4))
        sml = ctx.enter_context(tc.tile_pool(name="sml", bufs=2))
        attp = ctx.enter_context(tc.tile_pool(name="attp", bufs=1))
        osp = ctx.enter_context(tc.tile_pool(name="osp", bufs=2))
        ps = ctx.enter_context(tc.tile_pool(name="ps", bufs=1, space="PSUM"))
        dram = ctx.enter_context(tc.tile_pool(name="dram", bufs=1, space="DRAM"))

        # ---- constants / small weights ----
        identity = const.tile([P, P], FP32, tag="ident")
        make_identity(nc, identity)
        ones_f32 = const.tile([P, 64], FP32, tag="ones_f32")
        nc.vector.memset(ones_f32[:], 1.0)
        ones_col = const.tile([1, 64], MDT, tag="ones_col")   # bcast lhsT
        nc.vector.tensor_copy(ones_col[:], ones_f32[0:1, :])
        mb_sb = const.tile([P, B, NKT], FP32, tag="mb")
        nc.sync.dma_start(out=mb_sb[:], in_=mb.rearrange("b p t -> p b t"))

        wq_sb = wts.tile([P, KT, CW], BF16, tag="wq")
        wk_sb = wts.tile([P, KT, CW], BF16, tag="wk")
        wv_sb = wts.tile([P, KT, CW], BF16, tag="wv")
        for w_sb, w in ((wq_sb, wq), (wk_sb, wk), (wv_sb, wv)):
            nc.sync.dma_start(out=w_sb[:], in_=w.rearrange("(t p) m -> p t m", p=P))

        a2a_in = dram.tile([NCORES * CW, RPC], PDT, tag="a2a_in")
        a2a_out = dram.tile([NCORES * CW, RPC], PDT, tag="a2a_out")

        attT = attp.tile([P, B * N], PDT, tag="attT")  # [128 cols, 4096 rows]

        # psum tag cycler for projection / out-proj tiles
        _ptags = ["simA", "simB", "pvA", "pvB"]
        _pidx = [0]

        def psum_tile(shape):
            t = ps.tile(shape, FP32, tag=_ptags[_pidx[0] % 4], name=f"pst{_pidx[0]}")
            _pidx[0] += 1
            return t

        qT = {}
        kT = {}
        v_sb = {}

        for rep in range(reps):
         # --------------- phase 1a: projections (both batches) -------------
         for b in range(B):
             xts = []
             for kt in range(KT):
                 xtile = xtp.tile([P, N], BF16, tag="xt")
                 nc.sync.dma_start(out=xtile[:], in_=xt[b, kt * P:(kt + 1) * P, :])
                 xts.append(xtile)

             qT[b] = qkp.tile([P, N], MDT, tag="qT", name=f"qT{rep}_{b}")
             kT[b] = qkp.tile([P, N], MDT, tag="kT", name=f"kT{rep}_{b}")
             vT = vtp.tile([P, N], FP32, tag="vT")

             for w_sb, dst in ((wq_sb, qT[b]), (wk_sb, kT[b]), (wv_sb, vT)):
                 for jh in range(NQH):
                     pj = psum_tile([P, QW])
                     for kt in range(KT):
                         for js in range(2):
                             nc.tensor.matmul(
                                 pj[:, js * 512:(js + 1) * 512],
                                 _r(w_sb[:, kt, :]),
                                 _r(xts[kt][:, jh * QW + js * 512:
                                            jh * QW + (js + 1) * 512]),
                                 start=(kt == 0), stop=(kt == KT - 1),
                             )
                     nc.vector.tensor_copy(dst[:, jh * QW:(jh + 1) * QW], pj[:])

             # transpose vT -> v [n, 130] via PE (ping-pong psum tags)
             # layout per key tile: [vA(64) | ones | vB(64) | ones]; the ones
             # column makes PV also accumulate the softmax denominator (row 64)
             v_sb[b] = vsp.tile([P, NKT, 130], PDT, tag="vsb", name=f"vsb{rep}_{b}")
             nc.vector.tensor_copy(v_sb[b][:, :, 64], ones_f32[:, 0:NKT])
             nc.vector.tensor_copy(v_sb[b][:, :, 129], ones_f32[:, 0:NKT])
             for t in range(NKT):
                 tp = ps.tile([P, P], FP32, tag=_ptags[t % 2])
                 nc.tensor.transpose(tp[:], vT[:, t * P:(t + 1) * P], identity[:])
                 nc.vector.tensor_copy(v_sb[b][:, t, 0:64], tp[:, 0:64])
                 nc.vector.tensor_copy(v_sb[b][:, t, 65:129], tp[:, 64:128])

         # Wo: load AFTER projections so it reuses xt slots during attention
         wo_sb = [xtp.tile([P, 2 * DIM], PDT, tag="xt", name=f"wo_sb{rep}_{i}") for i in range(4)]
         for w4 in range(4):
             nc.sync.dma_start(
                 out=wo_sb[w4][:].rearrange("p (t m) -> p t m", m=DIM),
                 in_=wo.rearrange("(t p) m -> p t m", p=P)[:, 2 * w4: 2 * w4 + 2, :])

         # ---------------- phase 1b: attention ------------------------------
         for b in range(B):
             for jh in range(NQH):
                 pvA = ps.tile([65, QW], FP32, tag="pvA")
                 pvB = ps.tile([65, QW], FP32, tag="pvB")
                 for t in range(NKT):
                     sA = ps.tile([P, QW], FP32, tag="simA")
                     sB = ps.tile([P, QW], FP32, tag="simB")
                     for js in range(2):
                         qs = slice(jh * QW + js * 512, jh * QW + (js + 1) * 512)
                         ss = slice(js * 512, (js + 1) * 512)
                         nc.tensor.matmul(sA[:, ss],
                                          _r(kT[b][0:64, t * P:(t + 1) * P]),
                                          _r(qT[b][0:64, qs]))
                         nc.tensor.matmul(sB[:, ss],
                                          _r(kT[b][64:128, t * P:(t + 1) * P]),
                                          _r(qT[b][64:128, qs]))
                     pA = pp.tile([P, QW], PDT, tag="pA")
                     pB = pp.tile([P, QW], PDT, tag="pB")
                     nc.scalar.activation(pA[:], sA[:],
                                          mybir.ActivationFunctionType.Exp,
                                          bias=mb_sb[:, b, t:t + 1], scale=SCALE)
                     nc.scalar.activation(pB[:], sB[:],
                                          mybir.ActivationFunctionType.Exp,
                                          bias=mb_sb[:, b, t:t + 1], scale=SCALE)
                     st, sp = (t == 0), (t == NKT - 1)
                     for js in range(2):
                         ss = slice(js * 512, (js + 1) * 512)
                         nc.tensor.matmul(pvA[:, ss], _r(v_sb[b][:, t, 0:65]),
                                          _r(pA[:, ss]), start=st, stop=sp)
                         nc.tensor.matmul(pvB[:, ss], _r(v_sb[b][:, t, 65:130]),
                                          _r(pB[:, ss]), start=st, stop=sp)
                 # normalize: att = pv[0:64] * (1/pv[64]) (bcast via K=1 matmul)
                 span = slice(b * N + jh * QW, b * N + (jh + 1) * QW)
                 for h, pv in enumerate((pvA, pvB)):
                     rc = sml.tile([1, QW], MDT, tag="rc")
                     nc.vector.reciprocal(rc[:], pv[64:65, :])
                     bc = ps.tile([64, QW], FP32, tag=_ptags[h],
                                  name=f"bc{rep}_{b}{jh}{h}")
                     for js in range(2):
                         ss = slice(js * 512, (js + 1) * 512)
                         nc.tensor.matmul(bc[:, ss], _r(ones_col[:]),
                                          _r(rc[:, ss]))
                     bc_sb = sml.tile([64, QW], FP32, tag="bcs",
                                      name=f"bcs{rep}_{b}{jh}{h}")
                     nc.vector.tensor_copy(bc_sb[:], bc[:])
                     nc.vector.tensor_mul(attT[64 * h:64 * (h + 1), span],
                                          pv[0:64, :], bc_sb[:])

         # ---------------- all-to-all: head-parallel -> row-parallel --------
         nc.sync.dma_start(
             out=a2a_in.rearrange("(s p) f -> p s f", p=P),
             in_=attT[:].rearrange("p (s f) -> p s f", f=RPC))
         nc.gpsimd.collective_compute(
             "AllToAll", mybir.AluOpType.bypass,
             replica_groups=[list(range(NCORES))],
             ins=[a2a_in.opt()], outs=[a2a_out.opt()],
         )

         # [1024, 512] = att^T rows_c; load as two [128, 4, 512] tiles
         a2a_sb = []
         for half in range(2):
             tl = xtp.tile([P, 4 * RPC], PDT, tag="xt", name=f"a2a_sb{rep}_{half}")
             nc.sync.dma_start(
                 out=tl[:].rearrange("p (s f) -> p s f", f=RPC),
                 in_=a2a_out.rearrange("(s p) f -> p s f", p=P)[:, half * 4:
                                                                half * 4 + 4, :])
             a2a_sb.append(tl)

         # ---------------- phase 2: out-proj rows_c @ Wo --------------------
         out_sb = [osp.tile([P, 2 * DIM], FP32, tag="outsb", name=f"out_sb{rep}_{i}") for i in range(2)]
         for m in range(RPC // P):              # 4 row tiles
             po = psum_tile([P, DIM])
             for j in range(KT):                # 8 contraction tiles
                 src = a2a_sb[j // 4][:].rearrange("p (s f) -> p s f", f=RPC)
                 wsrc = wo_sb[j // 2][:].rearrange("p (t m) -> p t m", m=DIM)
                 for ns in range(2):
                     nc.tensor.matmul(
                         po[:, ns * 512:(ns + 1) * 512],
                         _r(src[:, j % 4, m * P:(m + 1) * P]),
                         _r(wsrc[:, j % 2, ns * 512:(ns + 1) * 512]),
                         start=(j == 0), stop=(j == KT - 1),
                     )
             nc.vector.tensor_copy(
                 out_sb[m // 2][:, (m % 2) * DIM:(m % 2 + 1) * DIM], po[:])
         for half in range(2):
             nc.sync.dma_start(
                 out=out.rearrange("(m p) d -> p m d", p=P)[:, half * 2:
                                                            half * 2 + 2, :],
                 in_=out_sb[half][:].rearrange("p (m d) -> p m d", d=DIM))

    nc.finalize()
    return nc


_CACHED = {}


def _get_program(reps=1):
    key = (MM_MODE, reps)
    if key not in _CACHED:
        _CACHED[key] = build_program(reps)
    return _CACHED[key]


def make_in_maps(x, mask, Wq, Wkv, Wo):
    """Host-side shard prep: per-core input dicts."""
    x = np.asarray(x, dtype=np.float32)
    mask = np.asarray(mask)
    Wq = np.asarray(Wq, dtype=np.float32)
    Wkv = np.asarray(Wkv, dtype=np.float32)
    Wo = np.asarray(Wo, dtype=np.float32)

    bf16 = __import__("ml_dtypes").bfloat16
    xT = np.ascontiguousarray(x.transpose(0, 2, 1)).astype(bf16)  # [B, DIM, N]
    mbias = np.where(mask, 0.0, MASK_NEG).astype(np.float32)   # [B, N]
    mbias = np.ascontiguousarray(
        mbias.reshape(B, NKT, P).transpose(0, 2, 1))           # [B, 128, NKT]

    in_maps = []
    for c in range(NCORES):
        cs = slice(c * CW, (c + 1) * CW)
        in_maps.append({
            "xt": xT,
            "wq": np.ascontiguousarray(Wq[:, cs]).astype(bf16),
            "wk": np.ascontiguousarray(Wkv[:, cs]).astype(bf16),
            "wv": np.ascontiguousarray(Wkv[:, DI + c * CW: DI + (c + 1) * CW]).astype(bf16),
            "wo": Wo.astype(__import__("ml_dtypes").bfloat16),
            "mb": mbias,
        })
    return in_maps


def assemble(results):
    outs = [np.asarray(results[c]["out"]) for c in range(NCORES)]
    return np.concatenate(outs, axis=0).reshape(B, N, DIM).astype(np.float32)


def kernel(x, mask, Wq, Wkv, Wo):
    from concourse.bass_utils import run_bass_kernel_spmd

    nc = _get_program()
    in_maps = make_in_maps(x, mask, Wq, Wkv, Wo)
    res = run_bass_kernel_spmd(nc, in_maps, list(range(NCORES)))
    return assemble(res.results)



# revision 22
# speedup vs baseline: 1.8340x; 1.8340x over previous
"""Distributed masked multi-head self-attention for Trainium2 (8 NeuronCores).

Problem: x:[2,2048,1024], mask:[2,2048], Wq:[1024,1024], Wkv:[1024,2048],
Wo:[1024,1024]  ->  out:[2,2048,1024]  (fp32)

Strategy (single SPMD launch, one NEFF on 8 cores):
  Head parallel: core c owns heads {2c, 2c+1} (128 contiguous columns of
  q/k/v).  Keys are COMPACTED on the host: only mask-valid key positions
  are shipped (padded up to a multiple of 128, KCAP), which cuts k/v
  projection, QK^T, exp and PV work by the masked-out fraction (~44% for
  a random half-dense mask).  Padding slots carry a -30000 bias so
  exp()=0 and they vanish from numerator and denominator alike.

  Per (b, jh) query block of 1024 the attention runs in "scores
  transposed" [key, query] layout:
      simT = kT_h as lhsT, qT as rhs -> [128 keys, 1024 q] in PSUM
      P    = exp(scale*simT + pad_bias)                     (ScalarE)
      pv[128,q] += [v_h | ones*64]^T @ P
  The 64 ones-columns replicate the softmax denominator across PSUM
  partitions 64:128 (extra lhsT columns are free: matmul time only
  depends on the moving free size), so normalization is a plain DVE
  reciprocal + multiply with no cross-partition broadcast.  v is
  projected directly into [key, dh] layout (x_k tiles as stationary
  operand), so no PE transposes are needed anywhere.

  Each finished [128 cols, 1024 q] block is AllToAll'd immediately (4
  chunked collectives overlap later attention blocks); each core ends up
  with att^T[:, its 128 rows] per chunk and runs the out-projection for
  those rows as "filler" PE work inside later attention loops (attention
  is exp/ACT-bound, the PE has slack there).

Precision tiers: x/xk/Wq/Wk/Wv/q/k/P/v/att/Wo bf16; PSUM accumulation,
softmax denominators and output fp32 (measured rel err ~5e-3, tol 2e-2).
"""

import os
import sys

import numpy as np

for _p in ("/opt/trn_rl_repo",):
    if _p not in sys.path and os.path.isdir(_p):
        sys.path.append(_p)

import concourse.bass as bass
from concourse import bacc
import concourse.mybir as mybir
import concourse.tile as tile
from contextlib import ExitStack

# ----- problem constants (hardcoded; kernel.py must be self-contained) -----
B, N, DIM, H, DH = 2, 2048, 1024, 16, 64
DI = H * DH                       # 1024
NCORES = 8
HPC = H // NCORES                 # 2 heads per core
CW = HPC * DH                     # 128 att columns per core
RPC = B * N // NCORES             # 512 output rows per core
SCALE = DH ** -0.5
MASK_NEG = -30000.0               # exp(scale*s + MASK_NEG) == 0 in fp32

P = 128                           # partitions
KT = DIM // P                     # 8 contraction tiles for projections
QW = 1024                         # query block width (exp batch width)
NQH = N // QW                     # 2 query blocks per batch
NCHUNK = B * NQH                  # 4 a2a chunks, 128 rows/core each
FP32 = mybir.dt.float32
F32R = mybir.dt.float32r
BF16 = mybir.dt.bfloat16


def build_program(reps=1, kcap=1152, collective=True):
    NKT = kcap // P               # key tiles per batch
    nc = bacc.Bacc(None, target_bir_lowering=False, num_devices=NCORES)

    xt = nc.dram_tensor("xt", [B, DIM, N], BF16, kind="ExternalInput")
    xk = nc.dram_tensor("xk", [B, DIM, kcap], BF16, kind="ExternalInput")
    wq = nc.dram_tensor("wq", [DIM, CW], BF16, kind="ExternalInput")
    wk = nc.dram_tensor("wk", [DIM, CW], BF16, kind="ExternalInput")
    wv = nc.dram_tensor("wv", [DIM, CW], BF16, kind="ExternalInput")
    wo = nc.dram_tensor("wo", [DI, DIM], BF16, kind="ExternalInput")
    mb = nc.dram_tensor("mb", [B, P, NKT], FP32, kind="ExternalInput")
    out = nc.dram_tensor("out", [RPC, DIM], FP32, kind="ExternalOutput")

    with tile.TileContext(nc) as tc, ExitStack() as ctx, \
            nc.allow_low_precision(reason="bf16 pipeline; psum accum stays fp32"):
        const = ctx.enter_context(tc.tile_pool(name="const", bufs=1))
        wts = ctx.enter_context(tc.tile_pool(name="wts", bufs=1))
        wop = ctx.enter_context(tc.tile_pool(name="wop", bufs=1))
        xtp = ctx.enter_context(tc.tile_pool(name="xtp", bufs=10))
        xkp = ctx.enter_context(tc.tile_pool(name="xkp", bufs=10))
        qkp = ctx.enter_context(tc.tile_pool(name="qkp", bufs=2))
        vsp = ctx.enter_context(tc.tile_pool(name="vsp", bufs=2))
        pp = ctx.enter_context(tc.tile_pool(name="pp", bufs=6))
        sml = ctx.enter_context(tc.tile_pool(name="sml", bufs=2))
        atb = ctx.enter_context(tc.tile_pool(name="atb", bufs=2))
        a2p = ctx.enter_context(tc.tile_pool(name="a2p", bufs=2))
        osp = ctx.enter_context(tc.tile_pool(name="osp", bufs=2))
        ps = ctx.enter_context(tc.tile_pool(name="ps", bufs=1, space="PSUM"))
        dram = ctx.enter_context(tc.tile_pool(name="dram", bufs=1, space="DRAM"))

        # ---- constants / weights ----
        mb_sb = const.tile([P, B, NKT], FP32, tag="mb")
        nc.sync.dma_start(out=mb_sb[:], in_=mb.rearrange("b p t -> p b t"))

        wq_sb = wts.tile([P, KT, CW], BF16, tag="wq")
        wk_sb = wts.tile([P, KT, CW], BF16, tag="wk")
        wv_sb = wts.tile([P, KT, CW], BF16, tag="wv")
        for w_sb, w in ((wq_sb, wq), (wk_sb, wk), (wv_sb, wv)):
            nc.sync.dma_start(out=w_sb[:], in_=w.rearrange("(t p) m -> p t m", p=P))
        wo_sb = wop.tile([P, KT, DIM], BF16, tag="wo")
        nc.scalar.dma_start(out=wo_sb[:], in_=wo.rearrange("(t p) m -> p t m", p=P))

        a2a_ins = [dram.tile([NCORES * CW, P], BF16, tag=f"a2a_in{k}",
                             name=f"a2a_in{k}") for k in range(NCHUNK)]
        a2a_outs = [dram.tile([NCORES * CW, P], BF16, tag=f"a2a_out{k}",
                              name=f"a2a_out{k}") for k in range(NCHUNK)]

        for rep in range(reps):
            qT = {}
            kT = {}
            v_sb = {}
            xts = {}
            xks = {}

            def load_x(b):
                xts[b] = []
                xks[b] = []
                for kt in range(KT):
                    xtile = xtp.tile([P, N], BF16, tag="xt")
                    nc.sync.dma_start(out=xtile[:], in_=xt[b, kt * P:(kt + 1) * P, :])
                    xts[b].append(xtile)
                for kt in range(KT):
                    xktile = xkp.tile([P, kcap], BF16, tag="xk")
                    nc.sync.dma_start(out=xktile[:],
                                      in_=xk[b, kt * P:(kt + 1) * P, :])
                    xks[b].append(xktile)

            def proj(b, r):
                qT[b] = qkp.tile([P, N], BF16, tag="qT", name=f"qT{r}_{b}")
                kT[b] = qkp.tile([P, kcap], BF16, tag="kT", name=f"kT{r}_{b}")
                # per (t, h): [v_h(64) | ones(64)]; the ones columns replicate
                # the softmax denominator across pv partitions 64:128
                v_sb[b] = vsp.tile([P, NKT, 2, P], BF16, tag="vsb",
                                   name=f"vsb{r}_{b}")
                nc.vector.memset(v_sb[b][:, :, :, 64:128], 1.0)
                # q-chunks ride the pv tags (idle until the first att block)
                # so they pipeline alongside k/v on the sim tags
                for jh in range(NQH):           # q: [128, 1024] chunks
                    pj = ps.tile([P, QW], FP32, tag=("pvA", "pvB")[jh % 2],
                                 name=f"qp{r}_{b}_{jh}")
                    for kt in range(KT):
                        for js in range(2):
                            ss = slice(js * 512, (js + 1) * 512)
                            nc.tensor.matmul(
                                pj[:, ss], wq_sb[:, kt, :],
                                xts[b][kt][:, jh * QW + js * 512:
                                           jh * QW + (js + 1) * 512],
                                start=(kt == 0), stop=(kt == KT - 1))
                    nc.vector.tensor_copy(qT[b][:, jh * QW:(jh + 1) * QW], pj[:])
                for c0 in range(0, kcap, QW):   # k: [128, <=1024] chunks
                    w = min(QW, kcap - c0)
                    pj = ps.tile([P, QW], FP32, tag=("simA", "simB")[(c0 // QW) % 2],
                                 name=f"kp{r}_{b}_{c0}")
                    for kt in range(KT):
                        for s0 in range(0, w, 512):
                            sw = min(512, w - s0)
                            nc.tensor.matmul(
                                pj[:, s0:s0 + sw], wk_sb[:, kt, :],
                                xks[b][kt][:, c0 + s0:c0 + s0 + sw],
                                start=(kt == 0), stop=(kt == KT - 1))
                    nc.vector.tensor_copy(kT[b][:, c0:c0 + w], pj[:, 0:w])
                for t0 in range(0, NKT, 4):     # v: direct [key, dh] layout
                    cnt = min(4, NKT - t0)
                    pj = ps.tile([P, 4, P], FP32, tag=("simA", "simB")[(t0 // 4) % 2],
                                 name=f"vp{r}_{b}_{t0}")
                    for i in range(cnt):
                        t = t0 + i
                        for kt in range(KT):
                            nc.tensor.matmul(
                                pj[:, i, :],
                                xks[b][kt][:, t * P:(t + 1) * P],
                                wv_sb[:, kt, :],
                                start=(kt == 0), stop=(kt == KT - 1))
                    for i in range(cnt):
                        t = t0 + i
                        nc.vector.tensor_copy(v_sb[b][:, t, 0, 0:64],
                                              pj[:, i, 0:64])
                        nc.vector.tensor_copy(v_sb[b][:, t, 1, 0:64],
                                              pj[:, i, 64:128])

            filler = []

            def pop_filler():
                if filler:
                    filler.pop(0)()

            def att_block(b, jh, chunk, r):
                while filler:       # phase-2 of the previous chunk runs here,
                    pop_filler()    # on the pv tags, before this block's claim
                pvA = ps.tile([P, QW], FP32, tag="pvA")
                pvB = ps.tile([P, QW], FP32, tag="pvB")
                for t in range(NKT):
                    sA = ps.tile([P, QW], FP32, tag="simA",
                                 name=f"sA{r}_{b}{jh}{t}")
                    sB = ps.tile([P, QW], FP32, tag="simB",
                                 name=f"sB{r}_{b}{jh}{t}")
                    for js in range(2):
                        qs = slice(jh * QW + js * 512, jh * QW + (js + 1) * 512)
                        ss = slice(js * 512, (js + 1) * 512)
                        nc.tensor.matmul(sA[:, ss],
                                         kT[b][0:64, t * P:(t + 1) * P],
                                         qT[b][0:64, qs])
                        nc.tensor.matmul(sB[:, ss],
                                         kT[b][64:128, t * P:(t + 1) * P],
                                         qT[b][64:128, qs])
                    pA = pp.tile([P, QW], BF16, tag="pA")
                    pB = pp.tile([P, QW], BF16, tag="pB")
                    nc.scalar.activation(pA[:], sA[:],
                                         mybir.ActivationFunctionType.Exp,
                                         bias=mb_sb[:, b, t:t + 1], scale=SCALE)
                    nc.scalar.activation(pB[:], sB[:],
                                         mybir.ActivationFunctionType.Exp,
                                         bias=mb_sb[:, b, t:t + 1], scale=SCALE)
                    st, sp = (t == 0), (t == NKT - 1)
                    for js in range(2):
                        ss = slice(js * 512, (js + 1) * 512)
                        nc.tensor.matmul(pvA[:, ss], v_sb[b][:, t, 0, :],
                                         pA[:, ss], start=st, stop=sp)
                        nc.tensor.matmul(pvB[:, ss], v_sb[b][:, t, 1, :],
                                         pB[:, ss], start=st, stop=sp)

                attT_blk = atb.tile([P, QW], BF16, tag="attT",
                                    name=f"attT{r}_{b}{jh}")
                for h, pv in enumerate((pvA, pvB)):
                    rc = sml.tile([64, QW], FP32, tag="rc")
                    nc.vector.reciprocal(rc[:], pv[64:128, :])
                    nc.vector.tensor_mul(attT_blk[64 * h:64 * (h + 1), :],
                                         pv[0:64, :], rc[:])

                # a2a this chunk: send [128 cols, 1024 q]; receive all 1024
                # cols for this core's 128 rows of the block
                nc.sync.dma_start(
                    out=a2a_ins[chunk].rearrange("(s p) f -> p s f", p=P),
                    in_=attT_blk[:].rearrange("p (s f) -> p s f", f=P))
                if collective:
                    nc.gpsimd.collective_compute(
                        "AllToAll", mybir.AluOpType.bypass,
                        replica_groups=[list(range(NCORES))],
                        ins=[a2a_ins[chunk].opt()], outs=[a2a_outs[chunk].opt()],
                    )
                else:  # timing-sim stand-in: local copy of the same volume
                    nc.sync.dma_start(out=a2a_outs[chunk][:], in_=a2a_ins[chunk][:])

            def phase2_thunks(chunk, r):
                """Out-proj filler: this core's 128 rows of chunk @ Wo."""
                state = {}

                def t_load():
                    a2s = a2p.tile([P, KT, P], BF16, tag="a2s",
                                   name=f"a2s{r}_{chunk}")
                    nc.sync.dma_start(
                        out=a2s[:],
                        in_=a2a_outs[chunk].rearrange("(s p) f -> p s f", p=P))
                    state["a2s"] = a2s

                def t_mm():
                    state["po"] = ps.tile([P, QW], FP32,
                                          tag=("pvA", "pvB")[chunk % 2],
                                          name=f"po{r}_{chunk}")
                    po, a2s = state["po"], state["a2s"]
                    for js in range(2):
                        ss = slice(js * 512, (js + 1) * 512)
                        for s in range(KT):
                            nc.tensor.matmul(po[:, ss], a2s[:, s, :],
                                             wo_sb[:, s, ss],
                                             start=(s == 0), stop=(s == KT - 1))

                def t_out():
                    osb = osp.tile([P, DIM], FP32, tag="outsb",
                                   name=f"osb{r}_{chunk}")
                    nc.vector.tensor_copy(osb[:], state["po"][:])
                    nc.sync.dma_start(
                        out=out[chunk * P:(chunk + 1) * P, :], in_=osb[:])

                return [t_load, t_mm, t_out]

            # ---------------- emission --------------------------------------
            load_x(0)
            proj(0, rep)
            load_x(1)
            proj(1, rep)
            att_block(0, 0, 0, rep)
            filler.extend(phase2_thunks(0, rep))
            att_block(0, 1, 1, rep)
            filler.extend(phase2_thunks(1, rep))
            att_block(1, 0, 2, rep)
            filler.extend(phase2_thunks(2, rep))
            att_block(1, 1, 3, rep)
            filler.extend(phase2_thunks(3, rep))
            while filler:
                pop_filler()

    nc.finalize()
    return nc


_CACHED = {}
_LAST_KCAP = [1152]


def _get_program(reps=1, kcap=None):
    if kcap is None:
        kcap = _LAST_KCAP[0]
    key = (reps, kcap)
    if key not in _CACHED:
        _CACHED[key] = build_program(reps, kcap=kcap)
    return _CACHED[key]


def _kcap_for(mask):
    cnt = int(np.asarray(mask).sum(axis=1).max())
    return max(P, ((cnt + P - 1) // P) * P)


def make_in_maps(x, mask, Wq, Wkv, Wo):
    """Host-side shard prep: per-core input dicts (keys compacted by mask)."""
    bf16 = __import__("ml_dtypes").bfloat16
    x = np.asarray(x, dtype=np.float32)
    mask = np.asarray(mask)
    Wq = np.asarray(Wq, dtype=np.float32)
    Wkv = np.asarray(Wkv, dtype=np.float32)
    Wo = np.asarray(Wo, dtype=np.float32)

    kcap = _kcap_for(mask)
    _LAST_KCAP[0] = kcap
    nkt = kcap // P

    xT = np.ascontiguousarray(x.transpose(0, 2, 1)).astype(bf16)  # [B, DIM, N]
    xkT = np.zeros((B, DIM, kcap), dtype=bf16)
    mbias = np.full((B, P, nkt), MASK_NEG, dtype=np.float32)
    for b in range(B):
        idx = np.nonzero(mask[b])[0]
        cnt = len(idx)
        xkT[b, :, :cnt] = xT[b][:, idx]
        valid = (np.arange(kcap) < cnt).reshape(nkt, P).T  # [P, nkt]
        mbias[b][valid] = 0.0

    in_maps = []
    for c in range(NCORES):
        cs = slice(c * CW, (c + 1) * CW)
        in_maps.append({
            "xt": xT,
            "xk": xkT,
            "wq": np.ascontiguousarray(Wq[:, cs]).astype(bf16),
            "wk": np.ascontiguousarray(Wkv[:, cs]).astype(bf16),
            "wv": np.ascontiguousarray(
                Wkv[:, DI + c * CW: DI + (c + 1) * CW]).astype(bf16),
            "wo": Wo.astype(bf16),
            "mb": mbias,
        })
    return in_maps


def assemble(results):
    # core c, chunk (b,jh) holds global rows b*2048 + jh*1024 + c*128 ..+128
    outs = np.stack([np.asarray(results[c]["out"]) for c in range(NCORES)])
    arr = outs.reshape(NCORES, B, NQH, P, DIM)
    return np.ascontiguousarray(
        arr.transpose(1, 2, 0, 3, 4).reshape(B, N, DIM)).astype(np.float32)


def kernel(x, mask, Wq, Wkv, Wo):
    from concourse.bass_utils import run_bass_kernel_spmd

    in_maps = make_in_maps(x, mask, Wq, Wkv, Wo)
    nc = _get_program()
    res = run_bass_kernel_spmd(nc, in_maps, list(range(NCORES)))
    return assemble(res.results)


# revision 31
# speedup vs baseline: 6.0758x; 3.3129x over previous
"""Distributed masked multi-head self-attention for Trainium2 (8 NeuronCores).

Problem: x:[2,2048,1024], mask:[2,2048], Wq:[1024,1024], Wkv:[1024,2048],
Wo:[1024,1024]  ->  out:[2,2048,1024]  (fp32)

Strategy (single SPMD launch, one NEFF on 8 cores):
  Head parallel: core c owns heads {2c, 2c+1} (128 contiguous columns of
  q/k/v).  Keys are COMPACTED on the host: only mask-valid key positions
  are shipped (padded up to a multiple of 128, KCAP), which cuts k/v
  projection, QK^T, exp and PV work by the masked-out fraction (~44% for
  a random half-dense mask).  Padding slots carry a -30000 bias so
  exp()=0 and they vanish from numerator and denominator alike.

  Per (b, jh) query block of 1024 the attention runs in "scores
  transposed" [key, query] layout:
      simT = kT_h as lhsT, qT as rhs -> [128 keys, 1024 q] in PSUM
      P    = exp(scale*simT + pad_bias)                     (ScalarE)
      pv[128,q] += [v_h | ones*64]^T @ P
  The 64 ones-columns replicate the softmax denominator across PSUM
  partitions 64:128 (extra lhsT columns are free: matmul time only
  depends on the moving free size), so normalization is a plain DVE
  reciprocal + multiply with no cross-partition broadcast.  v is
  projected directly into [key, dh] layout (x_k tiles as stationary
  operand), so no PE transposes are needed anywhere.

  Each finished [128 cols, 1024 q] block is AllToAll'd immediately (4
  chunked collectives overlap later attention blocks); each core ends up
  with att^T[:, its 128 rows] per chunk and runs the out-projection for
  those rows as "filler" PE work inside later attention loops (attention
  is exp/ACT-bound, the PE has slack there).

Precision tiers: x/xk/Wq/Wk/Wv/q/k/P/v/att/Wo bf16; PSUM accumulation,
softmax denominators and output fp32 (measured rel err ~5e-3, tol 2e-2).
"""

import os
import sys

import numpy as np

for _p in ("/opt/trn_rl_repo",):
    if _p not in sys.path and os.path.isdir(_p):
        sys.path.append(_p)

import concourse.bass as bass
from concourse import bacc
import concourse.mybir as mybir
import concourse.tile as tile
from contextlib import ExitStack

# ----- problem constants (hardcoded; kernel.py must be self-contained) -----
B, N, DIM, H, DH = 2, 2048, 1024, 16, 64
DI = H * DH                       # 1024
NCORES = 8
HPC = H // NCORES                 # 2 heads per core
CW = HPC * DH                     # 128 att columns per core
RPC = B * N // NCORES             # 512 output rows per core
SCALE = DH ** -0.5
MASK_NEG = -30000.0               # exp(scale*s + MASK_NEG) == 0 in fp32

P = 128                           # partitions
KT = DIM // P                     # 8 contraction tiles for projections
QW = 1024                         # query block width (exp batch width)
NQH = N // QW                     # 2 query blocks per batch
NCHUNK = B * NQH                  # 4 a2a chunks, 128 rows/core each
FP32 = mybir.dt.float32
F32R = mybir.dt.float32r
BF16 = mybir.dt.bfloat16


def build_program(reps=1, kcap=1152, collective=True):
    NKT = kcap // P               # key tiles per batch
    nc = bacc.Bacc(None, target_bir_lowering=False, num_devices=NCORES)

    warm = nc.dram_tensor("warm", [1, 4], FP32, kind="ExternalOutput")
    xt = nc.dram_tensor("xt", [B, DIM, N], BF16, kind="ExternalInput")
    xk = nc.dram_tensor("xk", [B, DIM, kcap], BF16, kind="ExternalInput")
    wq = nc.dram_tensor("wq", [DIM, CW], BF16, kind="ExternalInput")
    wk = nc.dram_tensor("wk", [DIM, CW], BF16, kind="ExternalInput")
    wv = nc.dram_tensor("wv", [DIM, CW], BF16, kind="ExternalInput")
    wo = nc.dram_tensor("wo", [DI, DIM], BF16, kind="ExternalInput")
    mb = nc.dram_tensor("mb", [B, P, NKT], FP32, kind="ExternalInput")
    out = nc.dram_tensor("out", [RPC, DIM], FP32, kind="ExternalOutput")

    with tile.TileContext(nc) as tc, ExitStack() as ctx, \
            nc.allow_low_precision(reason="bf16 pipeline; psum accum stays fp32"):
        const = ctx.enter_context(tc.tile_pool(name="const", bufs=1))
        wts = ctx.enter_context(tc.tile_pool(name="wts", bufs=1))
        wop = ctx.enter_context(tc.tile_pool(name="wop", bufs=1))
        xtp = ctx.enter_context(tc.tile_pool(name="xtp", bufs=10))
        xkp = ctx.enter_context(tc.tile_pool(name="xkp", bufs=10))
        qkp = ctx.enter_context(tc.tile_pool(name="qkp", bufs=2))
        vsp = ctx.enter_context(tc.tile_pool(name="vsp", bufs=2))
        pp = ctx.enter_context(tc.tile_pool(name="pp", bufs=6))
        sml = ctx.enter_context(tc.tile_pool(name="sml", bufs=2))
        atb = ctx.enter_context(tc.tile_pool(name="atb", bufs=2))
        a2p = ctx.enter_context(tc.tile_pool(name="a2p", bufs=2))
        osp = ctx.enter_context(tc.tile_pool(name="osp", bufs=2))
        ps = ctx.enter_context(tc.tile_pool(name="ps", bufs=1, space="PSUM"))
        dram = ctx.enter_context(tc.tile_pool(name="dram", bufs=1, space="DRAM"))

        # ---- constants / weights ----
        mb_sb = const.tile([P, B, NKT], FP32, tag="mb")
        nc.sync.dma_start(out=mb_sb[:], in_=mb.rearrange("b p t -> p b t"))

        wq_sb = wts.tile([P, KT, CW], BF16, tag="wq")
        wk_sb = wts.tile([P, KT, CW], BF16, tag="wk")
        wv_sb = wts.tile([P, KT, CW], BF16, tag="wv")
        for w_sb, w in ((wq_sb, wq), (wk_sb, wk), (wv_sb, wv)):
            nc.sync.dma_start(out=w_sb[:], in_=w.rearrange("(t p) m -> p t m", p=P))
        wo_sb = wop.tile([P, KT, DIM], BF16, tag="wo")
        nc.scalar.dma_start(out=wo_sb[:], in_=wo.rearrange("(t p) m -> p t m", p=P))

        a2a_ins = [dram.tile([NCORES * CW, P], BF16, tag=f"a2a_in{k}",
                             name=f"a2a_in{k}") for k in range(NCHUNK)]
        a2a_outs = [dram.tile([NCORES * CW, P], BF16, tag=f"a2a_out{k}",
                              name=f"a2a_out{k}") for k in range(NCHUNK)]

        # ---- PE warmup: ramp the tensor engine to full p-state while the
        # input DMAs land (a cold PE runs 2-4x slower for its first ~3us)
        wz = const.tile([P, 512], BF16, tag="wz")
        nc.vector.memset(wz[:], 0.0)
        wps = ps.tile([64, 512], FP32, tag="simA", name="warmps")
        for i in range(8):
            nc.tensor.matmul(wps[:], wz[:, 0:64], wz[:],
                             start=(i == 0), stop=(i == 7))
        wsb = const.tile([1, 4], FP32, tag="wsb")
        nc.vector.tensor_copy(wsb[:], wps[0:1, 0:4])
        nc.sync.dma_start(out=warm[:], in_=wsb[:])

        for rep in range(reps):
            qT = {}
            kT = {}
            v_sb = {}
            xts = {}
            xks = {}

            def load_x(b):
                xts[b] = []
                xks[b] = []
                for kt in range(KT):
                    xtile = xtp.tile([P, N], BF16, tag="xt")
                    nc.sync.dma_start(out=xtile[:], in_=xt[b, kt * P:(kt + 1) * P, :])
                    xts[b].append(xtile)
                for kt in range(KT):
                    xktile = xkp.tile([P, kcap], BF16, tag="xk")
                    nc.sync.dma_start(out=xktile[:],
                                      in_=xk[b, kt * P:(kt + 1) * P, :])
                    xks[b].append(xktile)

            tagc = [0]

            def next_tag(tags):
                tagc[0] += 1
                return tags[tagc[0] % len(tags)]

            def proj_chunks(b, r, tags, qw, vgrp):
                """Projection emission thunks for batch b; psum claims rotate
                over `tags`.  Batch 0 runs them inline with a 4-tag rotation
                (dense PE stream); batch 1 defers them as small fillers inside
                the attention t-loops."""
                qT[b] = qkp.tile([P, N], BF16, tag="qT", name=f"qT{r}_{b}")
                kT[b] = qkp.tile([P, kcap], BF16, tag="kT", name=f"kT{r}_{b}")
                # per (t, h): [v_h(64) | ones(64)]; the ones columns replicate
                # the softmax denominator across pv partitions 64:128
                v_sb[b] = vsp.tile([P, NKT, 2, P], BF16, tag="vsb",
                                   name=f"vsb{r}_{b}")
                nc.vector.memset(v_sb[b][:, :, :, 64:128], 1.0)
                th = []

                def q_chunk(c0, w):
                    def run():
                        pj = ps.tile([P, qw], FP32, tag=next_tag(tags),
                                     name=f"qp{r}_{b}_{c0}")
                        for kt in range(KT):
                            for s0 in range(0, w, 512):
                                nc.tensor.matmul(
                                    pj[:, s0:s0 + 512], wq_sb[:, kt, :],
                                    xts[b][kt][:, c0 + s0:c0 + s0 + 512],
                                    start=(kt == 0), stop=(kt == KT - 1))
                        nc.vector.tensor_copy(qT[b][:, c0:c0 + w], pj[:, 0:w])
                    return run

                def k_chunk(c0, w):
                    def run():
                        pj = ps.tile([P, qw], FP32, tag=next_tag(tags),
                                     name=f"kp{r}_{b}_{c0}")
                        for kt in range(KT):
                            for s0 in range(0, w, 512):
                                sw = min(512, w - s0)
                                nc.tensor.matmul(
                                    pj[:, s0:s0 + sw], wk_sb[:, kt, :],
                                    xks[b][kt][:, c0 + s0:c0 + s0 + sw],
                                    start=(kt == 0), stop=(kt == KT - 1))
                        nc.vector.tensor_copy(kT[b][:, c0:c0 + w], pj[:, 0:w])
                    return run

                def v_chunk(t0, cnt):
                    def run():
                        pj = ps.tile([P, vgrp, P], FP32, tag=next_tag(tags),
                                     name=f"vp{r}_{b}_{t0}")
                        for i in range(cnt):
                            t = t0 + i
                            for kt in range(KT):
                                nc.tensor.matmul(
                                    pj[:, i, :],
                                    xks[b][kt][:, t * P:(t + 1) * P],
                                    wv_sb[:, kt, :],
                                    start=(kt == 0), stop=(kt == KT - 1))
                        for i in range(cnt):
                            t = t0 + i
                            nc.vector.tensor_copy(v_sb[b][:, t, 0, 0:64],
                                                  pj[:, i, 0:64])
                            nc.vector.tensor_copy(v_sb[b][:, t, 1, 0:64],
                                                  pj[:, i, 64:128])
                    return run

                for c0 in range(0, N, qw):
                    th.append(q_chunk(c0, qw))
                for c0 in range(0, kcap, qw):
                    th.append(k_chunk(c0, min(qw, kcap - c0)))
                for t0 in range(0, NKT, vgrp):
                    th.append(v_chunk(t0, min(vgrp, NKT - t0)))
                return th

            filler = []

            def pop_filler():
                if filler:
                    filler.pop(0)()

            def att_block(b, jh, chunk, r):
                pvA = ps.tile([P, QW], FP32, tag="pvA")
                pvB = ps.tile([P, QW], FP32, tag="pvB")
                for t in range(NKT):
                    sA = ps.tile([P, QW], FP32, tag="simA",
                                 name=f"sA{r}_{b}{jh}{t}")
                    sB = ps.tile([P, QW], FP32, tag="simB",
                                 name=f"sB{r}_{b}{jh}{t}")
                    for js in range(2):
                        qs = slice(jh * QW + js * 512, jh * QW + (js + 1) * 512)
                        ss = slice(js * 512, (js + 1) * 512)
                        nc.tensor.matmul(sA[:, ss],
                                         kT[b][0:64, t * P:(t + 1) * P],
                                         qT[b][0:64, qs])
                        nc.tensor.matmul(sB[:, ss],
                                         kT[b][64:128, t * P:(t + 1) * P],
                                         qT[b][64:128, qs])
                    pA = pp.tile([P, QW], BF16, tag="pA")
                    pB = pp.tile([P, QW], BF16, tag="pB")
                    nc.scalar.activation(pA[:], sA[:],
                                         mybir.ActivationFunctionType.Exp,
                                         bias=mb_sb[:, b, t:t + 1], scale=SCALE)
                    nc.scalar.activation(pB[:], sB[:],
                                         mybir.ActivationFunctionType.Exp,
                                         bias=mb_sb[:, b, t:t + 1], scale=SCALE)
                    st, sp = (t == 0), (t == NKT - 1)
                    for js in range(2):
                        ss = slice(js * 512, (js + 1) * 512)
                        nc.tensor.matmul(pvA[:, ss], v_sb[b][:, t, 0, :],
                                         pA[:, ss], start=st, stop=sp)
                        nc.tensor.matmul(pvB[:, ss], v_sb[b][:, t, 1, :],
                                         pB[:, ss], start=st, stop=sp)
                    pop_filler()

                attT_blk = atb.tile([P, QW], BF16, tag="attT",
                                    name=f"attT{r}_{b}{jh}")
                for h, pv in enumerate((pvA, pvB)):
                    rc = sml.tile([64, QW], FP32, tag="rc")
                    nc.vector.reciprocal(rc[:], pv[64:128, :])
                    nc.vector.tensor_mul(attT_blk[64 * h:64 * (h + 1), :],
                                         pv[0:64, :], rc[:])

                # a2a this chunk: send [128 cols, 1024 q]; receive all 1024
                # cols for this core's 128 rows of the block
                nc.sync.dma_start(
                    out=a2a_ins[chunk].rearrange("(s p) f -> p s f", p=P),
                    in_=attT_blk[:].rearrange("p (s f) -> p s f", f=P))
                if collective:
                    nc.gpsimd.collective_compute(
                        "AllToAll", mybir.AluOpType.bypass,
                        replica_groups=[list(range(NCORES))],
                        ins=[a2a_ins[chunk].opt()], outs=[a2a_outs[chunk].opt()],
                    )
                else:  # timing-sim stand-in: local copy of the same volume
                    nc.sync.dma_start(out=a2a_outs[chunk][:], in_=a2a_ins[chunk][:])

            ph2_state = {}

            def ph2_load(chunk, r):
                """DMA-only filler: pull this core's rows of a delivered chunk
                into SBUF (popped one block after the chunk's collective)."""
                def run():
                    a2s = a2p.tile([P, KT, P], BF16, tag="a2s",
                                   name=f"a2s{r}_{chunk}")
                    nc.sync.dma_start(
                        out=a2s[:],
                        in_=a2a_outs[chunk].rearrange("(s p) f -> p s f", p=P))
                    ph2_state[chunk] = {"a2s": a2s}
                return run

            def ph2_compute(chunk, r):
                """Out-proj fillers (two psum halves + the out DMA)."""
                def half(js):
                    def run():
                        st = ph2_state[chunk]
                        if "osb" not in st:
                            st["osb"] = osp.tile([P, DIM], FP32, tag="outsb",
                                                 name=f"osb{r}_{chunk}")
                        po = ps.tile([P, 512], FP32,
                                     tag=next_tag(("simA", "simB")),
                                     name=f"po{r}_{chunk}_{js}")
                        ss = slice(js * 512, (js + 1) * 512)
                        for s in range(KT):
                            nc.tensor.matmul(po[:], st["a2s"][:, s, :],
                                             wo_sb[:, s, ss],
                                             start=(s == 0), stop=(s == KT - 1))
                        nc.vector.tensor_copy(st["osb"][:, ss], po[:])
                    return run

                def t_out():
                    nc.sync.dma_start(
                        out=out[chunk * P:(chunk + 1) * P, :],
                        in_=ph2_state[chunk]["osb"][:])

                return [half(0), half(1), t_out]

            # ---------------- emission --------------------------------------
            load_x(0)
            for th in proj_chunks(0, rep, ("simA", "simB", "pvA", "pvB"),
                                  qw=QW, vgrp=4):
                th()                  # batch-0 projections: dense 4-tag stream
            load_x(1)
            filler.extend(proj_chunks(1, rep, ("simA", "simB"), qw=512, vgrp=2))
            att_block(0, 0, 0, rep)
            filler.append(ph2_load(0, rep))
            att_block(0, 1, 1, rep)
            filler.append(ph2_load(1, rep))
            filler.extend(ph2_compute(0, rep))
            att_block(1, 0, 2, rep)
            filler.append(ph2_load(2, rep))
            filler.extend(ph2_compute(1, rep))
            att_block(1, 1, 3, rep)
            filler.append(ph2_load(3, rep))
            filler.extend(ph2_compute(2, rep))
            filler.extend(ph2_compute(3, rep))
            while filler:
                pop_filler()

    nc.finalize()
    return nc


_CACHED = {}
_LAST_KCAP = [1152]


def _get_program(reps=1, kcap=None):
    if kcap is None:
        kcap = _LAST_KCAP[0]
    key = (reps, kcap)
    if key not in _CACHED:
        _CACHED[key] = build_program(reps, kcap=kcap)
    return _CACHED[key]


def _kcap_for(mask):
    cnt = int(np.asarray(mask).sum(axis=1).max())
    return max(P, ((cnt + P - 1) // P) * P)


def make_in_maps(x, mask, Wq, Wkv, Wo):
    """Host-side shard prep: per-core input dicts (keys compacted by mask)."""
    bf16 = __import__("ml_dtypes").bfloat16
    x = np.asarray(x, dtype=np.float32)
    mask = np.asarray(mask)
    Wq = np.asarray(Wq, dtype=np.float32)
    Wkv = np.asarray(Wkv, dtype=np.float32)
    Wo = np.asarray(Wo, dtype=np.float32)

    kcap = _kcap_for(mask)
    _LAST_KCAP[0] = kcap
    nkt = kcap // P

    xT = np.ascontiguousarray(x.transpose(0, 2, 1)).astype(bf16)  # [B, DIM, N]
    xkT = np.zeros((B, DIM, kcap), dtype=bf16)
    mbias = np.full((B, P, nkt), MASK_NEG, dtype=np.float32)
    for b in range(B):
        idx = np.nonzero(mask[b])[0]
        cnt = len(idx)
        xkT[b, :, :cnt] = xT[b][:, idx]
        valid = (np.arange(kcap) < cnt).reshape(nkt, P).T  # [P, nkt]
        mbias[b][valid] = 0.0

    in_maps = []
    for c in range(NCORES):
        cs = slice(c * CW, (c + 1) * CW)
        in_maps.append({
            "xt": xT,
            "xk": xkT,
            "wq": np.ascontiguousarray(Wq[:, cs]).astype(bf16),
            "wk": np.ascontiguousarray(Wkv[:, cs]).astype(bf16),
            "wv": np.ascontiguousarray(
                Wkv[:, DI + c * CW: DI + (c + 1) * CW]).astype(bf16),
            "wo": Wo.astype(bf16),
            "mb": mbias,
        })
    return in_maps


def assemble(results):
    # core c, chunk (b,jh) holds global rows b*2048 + jh*1024 + c*128 ..+128
    outs = np.stack([np.asarray(results[c]["out"]) for c in range(NCORES)])
    arr = outs.reshape(NCORES, B, NQH, P, DIM)
    return np.ascontiguousarray(
        arr.transpose(1, 2, 0, 3, 4).reshape(B, N, DIM)).astype(np.float32)


def kernel(x, mask, Wq, Wkv, Wo):
    from concourse.bass_utils import run_bass_kernel_spmd

    in_maps = make_in_maps(x, mask, Wq, Wkv, Wo)
    nc = _get_program()
    res = run_bass_kernel_spmd(nc, in_maps, list(range(NCORES)))
    return assemble(res.results)
